# revision 4
# baseline (speedup 1.0000x reference)
"""Gaussian covariance kernel for Trainium2 (8 NeuronCores, SPMD).

Computes, per gaussian n:
    s = exp(scale[n])                  # [3]
    q = rot[n] / ||rot[n]||            # [4] quaternion (r,i,j,k)
    R = quat_to_rotmat(q)              # [3,3]
    Sigma[n] = (R*s) @ (R*s)^T         # [3,3]

Inputs : scale [4_000_000, 3] f32, rot [4_000_000, 4] f32
Output : [4_000_000, 3, 3] f32

Sharding: data-parallel over the gaussian dim across 8 cores
(500_000 each, padded to 500_096 = 128*3907 per core).

Math (scale-invariant, avoids the normalize):
    n2 = |q|^2 ; K = n2*I_part - 2*(quad products) so that R = K / n2
    w_j = (exp(s_j)/n2)^2 = exp(2*(s_j - ln n2))
    Sigma_ik = sum_j K_ij * K_kj * w_j
"""

import numpy as np

N_TOTAL = 4_000_000
N_CORES = 8
N_PER_CORE = N_TOTAL // N_CORES          # 500_000
P = 128
L = 3907                                 # ceil(500_000/128) -> pad to 128*3907
N_PAD = P * L                            # 500_096
F_TILE = 384                             # gaussians per partition per tile

_STATE = {}


def _build_kernel():
    import concourse.bass as bass
    import concourse.bacc as bacc
    import concourse.tile as tile
    from concourse import mybir

    f32 = mybir.dt.float32
    Alu = mybir.AluOpType
    Act = mybir.ActivationFunctionType

    nc = bacc.Bacc("TRN2", target_bir_lowering=False, debug=False,
                    num_devices=N_CORES)

    scale_d = nc.dram_tensor("scale", [P, L * 3], f32, kind="ExternalInput").ap()
    rot_d = nc.dram_tensor("rot", [P, L * 4], f32, kind="ExternalInput").ap()
    out_d = nc.dram_tensor("out", [P, L * 9], f32, kind="ExternalOutput").ap()

    # tile boundaries along the per-partition gaussian axis
    bounds = []
    t0 = 0
    while t0 < L:
        f = min(F_TILE, L - t0)
        bounds.append((t0, f))
        t0 += f

    with tile.TileContext(nc) as tc:
        with tc.tile_pool(name="io", bufs=2) as io, \
             tc.tile_pool(name="tmp", bufs=2) as tp:
            for (t0, F) in bounds:
                rot_t = io.tile([P, F, 4], f32, tag="rot")
                scl_t = io.tile([P, F, 3], f32, tag="scl")
                out_t = io.tile([P, F, 9], f32, tag="out")
                nc.sync.dma_start(out=rot_t[:, :, :],
                                  in_=rot_d[:, t0 * 4:(t0 + F) * 4]
                                  .rearrange("p (f c) -> p f c", c=4))
                nc.sync.dma_start(out=scl_t[:, :, :],
                                  in_=scale_d[:, t0 * 3:(t0 + F) * 3]
                                  .rearrange("p (f c) -> p f c", c=3))

                qr = rot_t[:, :, 0]
                qi = rot_t[:, :, 1]
                qj = rot_t[:, :, 2]
                qk = rot_t[:, :, 3]

                # squares (ACT): sq[:, :, c] = rot[:, :, c]^2
                sq_t = tp.tile([P, F, 4], f32, tag="sq")
                nc.scalar.activation(out=sq_t[:, :, :].rearrange("p f c -> p (f c)"),
                                     in_=rot_t[:, :, :].rearrange("p f c -> p (f c)"),
                                     func=Act.Square)
                d_ = sq_t[:, :, 0]
                a_ = sq_t[:, :, 1]
                b_ = sq_t[:, :, 2]
                c_ = sq_t[:, :, 3]

                # doubled products (POOL, fused *2): xy2 = 2*x*y
                ij = tp.tile([P, F], f32, tag="ij")
                kr = tp.tile([P, F], f32, tag="kr")
                ik = tp.tile([P, F], f32, tag="ik")
                jr = tp.tile([P, F], f32, tag="jr")
                jk = tp.tile([P, F], f32, tag="jk")
                ir = tp.tile([P, F], f32, tag="ir")
                nc.vector.scalar_tensor_tensor(out=ij, in0=qi, scalar=2.0, in1=qj,
                                               op0=Alu.mult, op1=Alu.mult)
                nc.vector.scalar_tensor_tensor(out=kr, in0=qk, scalar=2.0, in1=qr,
                                               op0=Alu.mult, op1=Alu.mult)
                nc.vector.scalar_tensor_tensor(out=ik, in0=qi, scalar=2.0, in1=qk,
                                               op0=Alu.mult, op1=Alu.mult)
                nc.vector.scalar_tensor_tensor(out=jr, in0=qj, scalar=2.0, in1=qr,
                                               op0=Alu.mult, op1=Alu.mult)
                nc.vector.scalar_tensor_tensor(out=jk, in0=qj, scalar=2.0, in1=qk,
                                               op0=Alu.mult, op1=Alu.mult)
                nc.vector.scalar_tensor_tensor(out=ir, in0=qi, scalar=2.0, in1=qr,
                                               op0=Alu.mult, op1=Alu.mult)

                # pair sums (POOL)
                ad = tp.tile([P, F], f32, tag="ad")
                bc = tp.tile([P, F], f32, tag="bc")
                ac = tp.tile([P, F], f32, tag="ac")
                ab = tp.tile([P, F], f32, tag="ab")
                nc.vector.tensor_add(out=ad, in0=d_, in1=a_)
                nc.vector.tensor_add(out=bc, in0=b_, in1=c_)
                nc.vector.tensor_add(out=ac, in0=a_, in1=c_)
                nc.vector.tensor_add(out=ab, in0=a_, in1=b_)

                n2 = tp.tile([P, F], f32, tag="n2")
                nc.vector.tensor_add(out=n2, in0=ad, in1=bc)

                # K matrix entries, R = K / n2
                K00 = tp.tile([P, F], f32, tag="K00")
                K11 = tp.tile([P, F], f32, tag="K11")
                K22 = tp.tile([P, F], f32, tag="K22")
                nc.vector.scalar_tensor_tensor(out=K00, in0=bc, scalar=-2.0, in1=n2,
                                               op0=Alu.mult, op1=Alu.add)
                nc.vector.scalar_tensor_tensor(out=K11, in0=ac, scalar=-2.0, in1=n2,
                                               op0=Alu.mult, op1=Alu.add)
                nc.vector.scalar_tensor_tensor(out=K22, in0=ab, scalar=-2.0, in1=n2,
                                               op0=Alu.mult, op1=Alu.add)

                K01 = tp.tile([P, F], f32, tag="K01")
                K10 = tp.tile([P, F], f32, tag="K10")
                K02 = tp.tile([P, F], f32, tag="K02")
                K20 = tp.tile([P, F], f32, tag="K20")
                K12 = tp.tile([P, F], f32, tag="K12")
                K21 = tp.tile([P, F], f32, tag="K21")
                nc.vector.tensor_sub(out=K01, in0=ij, in1=kr)
                nc.vector.tensor_add(out=K10, in0=ij, in1=kr)
                nc.vector.tensor_add(out=K02, in0=ik, in1=jr)
                nc.vector.tensor_sub(out=K20, in0=ik, in1=jr)
                nc.vector.tensor_sub(out=K12, in0=jk, in1=ir)
                nc.vector.tensor_add(out=K21, in0=jk, in1=ir)

                # w_j = exp(2*(s_j - ln n2))
                lg = tp.tile([P, F], f32, tag="lg")
                nc.scalar.activation(out=lg, in_=n2, func=Act.Ln)
                tm0 = tp.tile([P, F], f32, tag="tm0")
                tm1 = tp.tile([P, F], f32, tag="tm1")
                tm2 = tp.tile([P, F], f32, tag="tm2")
                nc.vector.tensor_sub(out=tm0, in0=scl_t[:, :, 0], in1=lg)
                nc.vector.tensor_sub(out=tm1, in0=scl_t[:, :, 1], in1=lg)
                nc.vector.tensor_sub(out=tm2, in0=scl_t[:, :, 2], in1=lg)
                w0 = tp.tile([P, F], f32, tag="w0")
                w1 = tp.tile([P, F], f32, tag="w1")
                w2 = tp.tile([P, F], f32, tag="w2")
                nc.scalar.activation(out=w0, in_=tm0, func=Act.Exp, scale=2.0)
                nc.scalar.activation(out=w1, in_=tm1, func=Act.Exp, scale=2.0)
                nc.scalar.activation(out=w2, in_=tm2, func=Act.Exp, scale=2.0)

                K = {(0, 0): K00, (0, 1): K01, (0, 2): K02,
                     (1, 0): K10, (1, 1): K11, (1, 2): K12,
                     (2, 0): K20, (2, 1): K21, (2, 2): K22}
                w = [w0, w1, w2]

                # C_ij = K_ij * w_j   (9 muls; 3 on POOL, 6 on DVE)
                C = {}
                pool_c = {(0, 0), (1, 0), (2, 0), (0, 1), (1, 1), (2, 1)}
                for i in range(3):
                    for j in range(3):
                        C[(i, j)] = tp.tile([P, F], f32, tag=f"C{i}{j}", name=f"C{i}{j}")
                        eng = nc.gpsimd if (i, j) in pool_c else nc.vector
                        eng.tensor_mul(out=C[(i, j)], in0=K[(i, j)], in1=w[j])

                # Sigma_ik = sum_j C_ij * K_kj  (6 unique entries)
                # products split: t1 on POOL, t2/t3 on DVE; adds on DVE
                for (i, k) in [(0, 0), (0, 1), (0, 2), (1, 1), (1, 2), (2, 2)]:
                    t1 = tp.tile([P, F], f32, tag="t1")
                    t2 = tp.tile([P, F], f32, tag="t2")
                    t3 = tp.tile([P, F], f32, tag="t3")
                    nc.gpsimd.tensor_mul(out=t1, in0=C[(i, 0)], in1=K[(k, 0)])
                    nc.gpsimd.tensor_mul(out=t2, in0=C[(i, 1)], in1=K[(k, 1)])
                    nc.vector.tensor_mul(out=t3, in0=C[(i, 2)], in1=K[(k, 2)])
                    s12 = tp.tile([P, F], f32, tag="s12")
                    nc.vector.tensor_add(out=s12, in0=t1, in1=t2)
                    nc.vector.tensor_add(out=out_t[:, :, 3 * i + k], in0=s12, in1=t3)

                # symmetric lower entries (ACT copies)
                for (i, k) in [(1, 0), (2, 0), (2, 1)]:
                    nc.scalar.copy(out=out_t[:, :, 3 * i + k],
                                   in_=out_t[:, :, 3 * k + i])

                nc.sync.dma_start(out=out_d[:, t0 * 9:(t0 + F) * 9]
                                  .rearrange("p (f c) -> p f c", c=9),
                                  in_=out_t[:, :, :])

    nc.compile()
    return nc


def _get_nc():
    if "nc" not in _STATE:
        _STATE["nc"] = _build_kernel()
    return _STATE["nc"]


def kernel(scale: np.ndarray, rot: np.ndarray) -> np.ndarray:
    from concourse.bass_utils import run_bass_kernel_spmd

    scale = np.asarray(scale, dtype=np.float32)
    rot = np.asarray(rot, dtype=np.float32)

    nc = _get_nc()

    in_maps = []
    for c in range(N_CORES):
        s = scale[c * N_PER_CORE:(c + 1) * N_PER_CORE]
        r = rot[c * N_PER_CORE:(c + 1) * N_PER_CORE]
        pad = N_PAD - N_PER_CORE
        s = np.concatenate([s, np.zeros((pad, 3), np.float32)], axis=0)
        rpad = np.zeros((pad, 4), np.float32)
        rpad[:, 0] = 1.0
        r = np.concatenate([r, rpad], axis=0)
        in_maps.append({
            "scale": np.ascontiguousarray(s.reshape(P, L * 3)),
            "rot": np.ascontiguousarray(r.reshape(P, L * 4)),
        })

    res = run_bass_kernel_spmd(nc, in_maps, core_ids=list(range(N_CORES)))

    outs = []
    for c in range(N_CORES):
        o = res.results[c]["out"].reshape(N_PAD, 9)[:N_PER_CORE]
        outs.append(o.reshape(N_PER_CORE, 3, 3))
    return np.concatenate(outs, axis=0)
